# revision 1
# baseline (speedup 1.0000x reference)
"""Causal self-attention kernel for 8 trn2 NeuronCores.

Sharding: 4 batches x 2 head-groups (8 heads each). Core c handles
batch c//2, heads (c%2)*8 .. +8. Host sums the two head-group partial
projections per batch and adds b_proj.

v3: all-bf16 (fp8 anywhere in the attention value path exceeds the
2e-2 gate: v-fp8 alone gives max y err 0.08, qk-fp8 0.11). PV uses a
128-column zero-padded stationary (64 v dims + ones col + 63 zeros) so
the weight load takes the fast FWL path (65-col stationary measured
333ns/matmul vs 254ns for 128-col).
- exp writes et bf16 (bias -ln16 kept: harmless, Z-consistent).
- causal masks bf16 on DVE (2-byte all-SBUF => 2x/4x DVE modes).
- software pipelining: qk matmuls of hp+1 (proj during hp3) are spliced
  between attention tiles via a work queue so PE rides out the ACT exp
  backlog instead of stalling on PSUM buffers.
- engine split: ACT = exp + Z-row; DVE = all psum evacuations + masks +
  normalize; Pool = broadcasts; PE = matmuls only.
"""

import sys
import os

for _p in ("/opt/trn_rl_repo", "/root/.axon_site/_ro/trn_rl_repo"):
    if os.path.isdir(_p) and _p not in sys.path:
        sys.path.insert(0, _p)

import numpy as np
import ml_dtypes
import concourse.bass as bass  # noqa: F401
import concourse.mybir as mybir
import concourse.tile as tile
from concourse import bacc, bass_utils

F32 = mybir.dt.float32
BF16 = mybir.dt.bfloat16
FP8 = mybir.dt.float8e4  # unused in v3
ActF = mybir.ActivationFunctionType
DR = mybir.MatmulPerfMode.DoubleRow
AluOp = mybir.AluOpType

B, S, D, H = 4, 2048, 1024, 16
NH = 8          # heads per core
HPAIRS = NH // 2
KT = D // 128   # 8 k-tiles over D
N_CORES = 8
EXPB = -2.772588722239781  # -ln(16): exp(s/8)/16 keeps et <= ~15 in fp8

_nc_cache = {}


def build_nc(S_tok=S, n_cores=N_CORES):
    key = (S_tok, n_cores)
    if key in _nc_cache:
        return _nc_cache[key]
    IC = S_tok // 512      # query chunks
    NT = S_tok // 128      # token tiles
    nc = bacc.Bacc("TRN2", target_bir_lowering=False, debug=False,
                   num_devices=n_cores)
    xT = nc.dram_tensor("xT", [D, S_tok], BF16, kind="ExternalInput").ap()
    Wq = nc.dram_tensor("Wq", [D, 512], BF16, kind="ExternalInput").ap()
    Wk = nc.dram_tensor("Wk", [D, 512], BF16, kind="ExternalInput").ap()
    Wv = nc.dram_tensor("Wv", [D, 512], BF16, kind="ExternalInput").ap()
    bq = nc.dram_tensor("bq", [512, 1], F32, kind="ExternalInput").ap()
    bk = nc.dram_tensor("bk", [512, 1], F32, kind="ExternalInput").ap()
    bv = nc.dram_tensor("bv", [1, 512], F32, kind="ExternalInput").ap()
    Wp = nc.dram_tensor("Wp", [512, D], BF16, kind="ExternalInput").ap()
    out = nc.dram_tensor("out", [S_tok, D], F32, kind="ExternalOutput").ap()

    with tile.TileContext(nc) as tc:
        with tc.tile_pool(name="pp", bufs=1) as pp, \
             tc.tile_pool(name="hs", bufs=1) as hs, \
             tc.tile_pool(name="ps_s", bufs=2, space="PSUM") as ps_s, \
             tc.tile_pool(name="ps_y", bufs=1, space="PSUM") as ps_y, \
             tc.tile_pool(name="ps_m", bufs=2, space="PSUM") as ps_m:
            # ---- persistent tiles ----
            xtr = [pp.tile([128, S_tok], BF16, name=f"xtr{k}")
                   for k in range(KT)]
            # v in fp8, [slot(2), head(8), 64 dims + ones col, pad to 80];
            # memset(1.0) up front makes col 64 the ones column
            v8 = [pp.tile([128, 2, NH, 128], BF16, name=f"v8_{g}")
                  for g in range(NT // 2)]
            for g in range(NT // 2):
                nc.gpsimd.memset(v8[g], 0.0)
                nc.gpsimd.memset(v8[g][:, :, :, 64:65], 1.0)
            # yT per (hp, ic): [128 = 2 heads x 64 dims, 512 queries] bf16
            yT = [[pp.tile([128, 512], BF16, name=f"yt{hp}_{ic}")
                   for ic in range(IC)] for hp in range(HPAIRS)]
            # causal masks fp8, duplicated per head: [128, 2, w]
            masks = []
            for t in range(4):
                wm = 128 * (t + 1)
                mbf = pp.tile([128, wm], BF16, name=f"maskb{t}")
                nc.gpsimd.memset(mbf, 1.0)
                nc.gpsimd.affine_select(
                    out=mbf, in_=mbf, compare_op=AluOp.is_ge,
                    fill=0.0, base=-128 * t, pattern=[[1, wm]],
                    channel_multiplier=-1)
                m8 = pp.tile([128, 2, wm], BF16, name=f"mask8_{t}")
                nc.vector.tensor_copy(m8[:, 0:1, :], mbf)
                nc.vector.tensor_copy(m8[:, 1:2, :], mbf)
                masks.append(m8)
            expb = pp.tile([128, 1], F32, name="expb")
            nc.gpsimd.memset(expb, EXPB)
            bvb = pp.tile([128, 512], F32, name="bvb")
            wp = [pp.tile([128, D], BF16, name=f"wp{k}")
                  for k in range(HPAIRS)]

            def fetch_w(hp):
                wq_t, wk_t = [], []
                for k in range(KT):
                    tq = hs.tile([128, 128], BF16, tag=f"wq{k}", bufs=2,
                                 name="wq")
                    nc.sync.dma_start(
                        tq, Wq[k * 128:(k + 1) * 128,
                               hp * 128:(hp + 1) * 128])
                    wq_t.append(tq)
                    tk = hs.tile([128, 128], BF16, tag=f"wk{k}", bufs=2,
                                 name="wk")
                    nc.sync.dma_start(
                        tk, Wk[k * 128:(k + 1) * 128,
                               hp * 128:(hp + 1) * 128])
                    wk_t.append(tk)
                bqt = hs.tile([128, 1], F32, tag="bq", bufs=2, name="bqt")
                nc.sync.dma_start(bqt, bq[hp * 128:(hp + 1) * 128, 0:1])
                bkt = hs.tile([128, 1], F32, tag="bk", bufs=2, name="bkt")
                nc.sync.dma_start(bkt, bk[hp * 128:(hp + 1) * 128, 0:1])
                return wq_t, wk_t, bqt, bkt

            # ---- work queue for PE-splice thunks ----
            pending = []

            def drain(ns_budget):
                spent = 0
                while pending and spent < ns_budget:
                    cost, fn = pending.pop(0)
                    fn()
                    spent += cost

            def drain_all():
                drain(1 << 60)

            def push_qk_group(w, bias_t, dst, c):
                # one psq group: 8 accumulating matmuls + DVE evac w/ bias
                state = {}

                def mk_mm(k):
                    def f():
                        if "ps" not in state:
                            state["ps"] = ps_m.tile([128, 512], F32,
                                                    tag="m512", name="psq")
                        nc.tensor.matmul(
                            state["ps"], w[k],
                            xtr[k][:, c * 512:(c + 1) * 512],
                            start=(k == 0), stop=(k == KT - 1))
                    return f

                for k in range(KT):
                    pending.append((213, mk_mm(k)))

                def evac():
                    nc.vector.tensor_scalar_add(
                        dst[:, c * 512:(c + 1) * 512], state["ps"], bias_t)
                pending.append((0, evac))

            def push_proj_group(ic):
                # proj for the 4 token tiles of query chunk ic
                for tt4 in range(4):
                    state = {}

                    def mk_mm(k, nch, tt4=tt4, state=state):
                        def f():
                            tg = f"mo{nch}"
                            if tg not in state:
                                state[tg] = ps_m.tile([128, 512], F32,
                                                      tag="m512", name="pso")
                            nc.tensor.matmul(
                                state[tg],
                                yT[k][ic][:, tt4 * 128:(tt4 + 1) * 128],
                                wp[k][:, nch * 512:(nch + 1) * 512],
                                start=(k == 0), stop=(k == HPAIRS - 1))
                        return f

                    def mk_evac(nch, tt4=tt4, state=state):
                        def f():
                            tt = ic * 4 + tt4
                            ot = hs.tile([128, 512], F32, tag="ot", bufs=3,
                                         name="ot")
                            nc.vector.tensor_copy(ot, state[f"mo{nch}"])
                            nc.sync.dma_start(
                                out[tt * 128:(tt + 1) * 128,
                                    nch * 512:(nch + 1) * 512], ot)
                        return f

                    for k in range(HPAIRS):
                        for nch in range(2):
                            pending.append((213, mk_mm(k, nch)))
                    for nch in range(2):
                        pending.append((0, mk_evac(nch)))

            # ---- phase V + qk(0): V matmuls interleaved with hp0 q/k ----
            qts = {}
            with tc.tile_pool(name="wvp", bufs=1) as wvp:
                bvr = wvp.tile([1, 512], F32, name="bvr")
                nc.sync.dma_start(bvr, bv)
                nc.gpsimd.partition_broadcast(bvb, bvr)
                wv = [wvp.tile([128, 512], BF16, name=f"wv{k}")
                      for k in range(KT)]
                for k in range(KT):
                    nc.sync.dma_start(wv[k], Wv[k * 128:(k + 1) * 128, :])
                w0 = fetch_w(0)
                qt0 = hs.tile([128, S_tok], BF16, tag="qt", bufs=2,
                              name="qt")
                kt0 = hs.tile([128, S_tok], BF16, tag="kt", bufs=2,
                              name="kt")
                qts[0] = (qt0, kt0)
                for c in range(IC):
                    cs = slice(c * 512, (c + 1) * 512)
                    for k in range(KT):
                        nc.sync.dma_start(xtr[k][:, cs],
                                          xT[k * 128:(k + 1) * 128, cs])
                for c in range(IC):
                    for j in range(4):
                        t = c * 4 + j
                        psv = ps_m.tile([128, 512], F32, tag="m512",
                                        name="psv")
                        for k in range(KT):
                            nc.tensor.matmul(
                                psv, xtr[k][:, t * 128:(t + 1) * 128],
                                wv[k], start=(k == 0), stop=(k == KT - 1))
                        nc.vector.tensor_add(
                            v8[t // 2][:, t % 2:t % 2 + 1, :, 0:64],
                            psv.rearrange("p (h c) -> p h c", c=64),
                            bvb.rearrange("p (h c) -> p h c", c=64))
                    push_qk_group(w0[0], w0[2], qt0, c)
                    push_qk_group(w0[1], w0[3], kt0, c)
                    drain_all()

            # ---- attention per hp, with spliced qk(hp+1)/proj ----
            for hp in range(HPAIRS):
                qt, kt_ = qts[hp]
                if hp + 1 < HPAIRS:
                    wn = fetch_w(hp + 1)
                    qtn = hs.tile([128, S_tok], BF16, tag="qt", bufs=2,
                                  name="qt")
                    ktn = hs.tile([128, S_tok], BF16, tag="kt", bufs=2,
                                  name="kt")
                    qts[hp + 1] = (qtn, ktn)
                if hp == HPAIRS - 1:
                    for k in range(HPAIRS):
                        nc.sync.dma_start(wp[k],
                                          Wp[k * 128:(k + 1) * 128, :])
                for ic in range(IC):
                    if hp + 1 < HPAIRS:
                        push_qk_group(wn[0], wn[2], qts[hp + 1][0], ic)
                        push_qk_group(wn[1], wn[3], qts[hp + 1][1], ic)
                    psy = [ps_y.tile([128, 512], F32, tag=f"psy{h2}",
                                     name=f"psy{h2}") for h2 in range(2)]
                    for g in range(2 * ic + 2):
                        for s2 in range(2):
                            jt = 2 * g + s2
                            pss = ps_s.tile([128, 1024], F32, tag="pss",
                                            name="pss")
                            nc.tensor.matmul(
                                pss[:, 0:512],
                                kt_[0:64, jt * 128:(jt + 1) * 128],
                                qt[0:64, ic * 512:(ic + 1) * 512],
                                start=True, stop=True,
                                tile_position=(0, 0))
                            nc.tensor.matmul(
                                pss[:, 512:1024],
                                kt_[64:128, jt * 128:(jt + 1) * 128],
                                qt[64:128, ic * 512:(ic + 1) * 512],
                                start=True, stop=True,
                                tile_position=(64, 0))
                            et = hs.tile([128, 2, 512], BF16, tag="et",
                                         bufs=6, name="et")
                            nc.scalar.activation(
                                et, pss, ActF.Exp, bias=expb, scale=0.125)
                            tdx = jt - 4 * ic
                            if tdx >= 0:
                                w_ = 128 * (tdx + 1)
                                ev = et[:, :, 0:w_]
                                nc.vector.tensor_mul(ev, ev, masks[tdx])
                            for h2 in range(2):
                                nc.tensor.matmul(
                                    psy[h2],
                                    v8[g][:, s2:s2 + 1,
                                          2 * hp + h2:2 * hp + h2 + 1, :],
                                    et[:, h2:h2 + 1, :],
                                    start=(jt == 0),
                                    stop=(jt == 4 * ic + 3))
                            drain(500)
                    # ---- evacuate + normalize this ic ----
                    zc = hs.tile([1, 1024], F32, tag="zc", bufs=2,
                                 name="zc")
                    for h2 in range(2):
                        t65 = hs.tile([64, 512], BF16, tag="t65", bufs=2,
                                      name="t65")
                        nc.vector.tensor_copy(t65, psy[h2][0:64, :])
                        nc.sync.dma_start(
                            yT[hp][ic][h2 * 64:(h2 + 1) * 64, :], t65)
                        nc.scalar.activation(
                            zc[0:1, h2 * 512:(h2 + 1) * 512],
                            psy[h2][64:65, :], ActF.Copy)
                    zs = hs.tile([128, 8], F32, tag="zs", bufs=2,
                                 name="zs")
                    nc.sync.dma_start(zs, zc)
                    nc.vector.reciprocal(zs, zs)
                    nc.sync.dma_start(zc, zs)
                    bcf = hs.tile([128, 512], F32, tag="bcf", bufs=2,
                                  name="bcf")
                    nc.gpsimd.partition_broadcast(bcf, zc[0:1, 512:1024])
                    nc.gpsimd.partition_broadcast(bcf[0:64, :],
                                                  zc[0:1, 0:512])
                    nc.vector.tensor_mul(yT[hp][ic], yT[hp][ic], bcf)
                    if hp == HPAIRS - 1 and ic >= 1:
                        push_proj_group(ic - 1)
                drain_all()
            push_proj_group(IC - 1)
            drain_all()
    nc.finalize()
    _nc_cache[key] = nc
    return nc


def make_in_maps(x, W_attn, b_attn, W_proj):
    Bx, Sx, Dx = x.shape
    bf = ml_dtypes.bfloat16
    in_maps = []
    xt_b = [np.ascontiguousarray(x[b].T).astype(bf) for b in range(Bx)]
    for c in range(N_CORES):
        b = c // 2
        g = c % 2
        cs = slice(g * 512, (g + 1) * 512)
        in_maps.append({
            "xT": xt_b[b],
            "Wq": np.ascontiguousarray(W_attn[:, 0:D][:, cs]).astype(bf),
            "Wk": np.ascontiguousarray(W_attn[:, D:2 * D][:, cs]).astype(bf),
            "Wv": np.ascontiguousarray(W_attn[:, 2 * D:3 * D][:, cs]).astype(bf),
            "bq": np.ascontiguousarray(
                b_attn[0:D][cs][:, None]).astype(np.float32),
            "bk": np.ascontiguousarray(
                b_attn[D:2 * D][cs][:, None]).astype(np.float32),
            "bv": np.ascontiguousarray(
                b_attn[2 * D:3 * D][cs][None, :]).astype(np.float32),
            "Wp": np.ascontiguousarray(W_proj[cs, :]).astype(bf),
        })
    return in_maps


def kernel(x, W_attn, b_attn, W_proj, b_proj, trace=False):
    x = np.asarray(x, dtype=np.float32)
    W_attn = np.asarray(W_attn, dtype=np.float32)
    b_attn = np.asarray(b_attn, dtype=np.float32)
    W_proj = np.asarray(W_proj, dtype=np.float32)
    b_proj = np.asarray(b_proj, dtype=np.float32)
    nc = build_nc(x.shape[1], N_CORES)
    in_maps = make_in_maps(x, W_attn, b_attn, W_proj)
    res = bass_utils.run_bass_kernel_spmd(
        nc, in_maps, core_ids=list(range(N_CORES)), trace=trace)
    Bx, Sx, Dx = x.shape
    outp = np.empty((Bx, Sx, Dx), dtype=np.float32)
    for b in range(Bx):
        outp[b] = (res.results[2 * b]["out"] + res.results[2 * b + 1]["out"]
                   + b_proj[None, :])
    if trace:
        return outp, res
    return outp



# revision 6
# speedup vs baseline: 1.0728x; 1.0728x over previous
"""Causal self-attention kernel for 8 trn2 NeuronCores.

Sharding: 4 batches x 2 head-groups (8 heads each). Core c handles
batch c//2, heads (c%2)*8 .. +8. Host sums the two head-group partial
projections per batch and adds b_proj.

v4 (from v3 @ 318us):
- diagonal trim: QK/exp/PV restricted to the causally-valid query range
  [off:512] per key tile (off = 128*tdx); one shared [128,2,128]
  triangle mask replaces the 4 wide masks; diag tiles issued in order
  [tdx1,2,3,0] so the last PV write is full-width (clean PSUM stop
  and start=True covers the whole bank).
- psy for odd heads has v-dims in stationary cols 64:127 and the ones
  (Z) column at col 0, so both head evacuations psy->yT are
  same-partition DVE copies (no SBUF->SBUF DMA).
- startup: weight/bias DMAs issue on the scalar HWDGE queue in
  parallel with xtr chunk DMAs on sync; v8 memsets only cover the
  pad+ones region.
- v3 retained: all-bf16, exp(s/8)/16 bias, PE work-queue splicing of
  qk(hp+1)/proj between attention tiles.
"""

import sys
import os

for _p in ("/opt/trn_rl_repo", "/root/.axon_site/_ro/trn_rl_repo"):
    if os.path.isdir(_p) and _p not in sys.path:
        sys.path.insert(0, _p)

import numpy as np
import ml_dtypes
import concourse.bass as bass  # noqa: F401
import concourse.mybir as mybir
import concourse.tile as tile
from concourse import bacc, bass_utils

F32 = mybir.dt.float32
BF16 = mybir.dt.bfloat16
ActF = mybir.ActivationFunctionType
AluOp = mybir.AluOpType

B, S, D, H = 4, 2048, 1024, 16
NH = 8          # heads per core
HPAIRS = NH // 2
KT = D // 128   # 8 k-tiles over D
N_CORES = 8
EXPB = -2.772588722239781  # -ln(16): exp(s/8)/16 keeps et small

_nc_cache = {}


def build_nc(S_tok=S, n_cores=N_CORES):
    key = (S_tok, n_cores)
    if key in _nc_cache:
        return _nc_cache[key]
    IC = S_tok // 512      # query chunks
    NT = S_tok // 128      # token tiles
    nc = bacc.Bacc("TRN2", target_bir_lowering=False, debug=False,
                   num_devices=n_cores)
    xT = nc.dram_tensor("xT", [D, S_tok], BF16, kind="ExternalInput").ap()
    Wq = nc.dram_tensor("Wq", [D, 512], BF16, kind="ExternalInput").ap()
    Wk = nc.dram_tensor("Wk", [D, 512], BF16, kind="ExternalInput").ap()
    Wv = nc.dram_tensor("Wv", [D, 512], BF16, kind="ExternalInput").ap()
    bq = nc.dram_tensor("bq", [512, 1], F32, kind="ExternalInput").ap()
    bk = nc.dram_tensor("bk", [512, 1], F32, kind="ExternalInput").ap()
    bv = nc.dram_tensor("bv", [1, 512], F32, kind="ExternalInput").ap()
    Wp = nc.dram_tensor("Wp", [512, D], BF16, kind="ExternalInput").ap()
    out = nc.dram_tensor("out", [S_tok, D], F32, kind="ExternalOutput").ap()

    with tile.TileContext(nc) as tc:
        with tc.tile_pool(name="pp", bufs=1) as pp, \
             tc.tile_pool(name="hs", bufs=1) as hs, \
             tc.tile_pool(name="ps_s", bufs=2, space="PSUM") as ps_s, \
             tc.tile_pool(name="ps_y", bufs=1, space="PSUM") as ps_y, \
             tc.tile_pool(name="ps_m", bufs=2, space="PSUM") as ps_m:
            # ---- persistent tiles ----
            xtr = [pp.tile([128, S_tok], BF16, name=f"xtr{k}")
                   for k in range(KT)]
            # v stationaries: [slot(2), parity(2), hpair(4), 128].
            # even heads (parity 0): v dims cols 0:64, ones col 64,
            #   pad 65:128 -> psy partitions 0:64 = y, 64 = Z.
            # odd heads  (parity 1): ones col 0, pad 1:64, v dims
            #   cols 64:128 -> psy partitions 0 = Z, 64:128 = y.
            v8 = [pp.tile([128, 2, 2, HPAIRS, 128], BF16, name=f"v8_{g}")
                  for g in range(NT // 2)]
            for g in range(NT // 2):
                nc.gpsimd.memset(v8[g][:, :, 0:1, :, 64:128], 0.0)
                nc.gpsimd.memset(v8[g][:, :, 1:2, :, 0:64], 0.0)
                nc.gpsimd.memset(v8[g][:, :, 0:1, :, 64:65], 1.0)
                nc.gpsimd.memset(v8[g][:, :, 1:2, :, 0:1], 1.0)
            # yT per (hp, ic): [128 = 2 heads x 64 dims, 512 queries] bf16
            yT = [[pp.tile([128, 512], BF16, name=f"yt{hp}_{ic}")
                   for ic in range(IC)] for hp in range(HPAIRS)]
            # single triangle mask [128, 2, 128]: keep where query >= key
            mbf = pp.tile([128, 128], BF16, name="maskb")
            nc.gpsimd.memset(mbf, 1.0)
            nc.gpsimd.affine_select(
                out=mbf, in_=mbf, compare_op=AluOp.is_ge,
                fill=0.0, base=0, pattern=[[1, 128]],
                channel_multiplier=-1)
            mask2 = pp.tile([128, 2, 128], BF16, name="mask2")
            nc.vector.tensor_copy(mask2[:, 0:1, :], mbf)
            nc.vector.tensor_copy(mask2[:, 1:2, :], mbf)
            expb = pp.tile([128, 1], F32, name="expb")
            nc.gpsimd.memset(expb, EXPB)
            bvb = pp.tile([128, 512], F32, name="bvb")
            wp = [pp.tile([128, D], BF16, name=f"wp{k}")
                  for k in range(HPAIRS)]

            def fetch_w(hp):
                wq_t, wk_t = [], []
                for k in range(KT):
                    tq = hs.tile([128, 128], BF16, tag=f"wq{k}", bufs=2,
                                 name="wq")
                    nc.scalar.dma_start(
                        tq, Wq[k * 128:(k + 1) * 128,
                               hp * 128:(hp + 1) * 128])
                    wq_t.append(tq)
                    tk = hs.tile([128, 128], BF16, tag=f"wk{k}", bufs=2,
                                 name="wk")
                    nc.scalar.dma_start(
                        tk, Wk[k * 128:(k + 1) * 128,
                               hp * 128:(hp + 1) * 128])
                    wk_t.append(tk)
                bqt = hs.tile([128, 1], F32, tag="bq", bufs=2, name="bqt")
                nc.scalar.dma_start(bqt, bq[hp * 128:(hp + 1) * 128, 0:1])
                bkt = hs.tile([128, 1], F32, tag="bk", bufs=2, name="bkt")
                nc.scalar.dma_start(bkt, bk[hp * 128:(hp + 1) * 128, 0:1])
                return wq_t, wk_t, bqt, bkt

            # ---- work queue for PE-splice thunks ----
            pending = []

            def drain(ns_budget):
                spent = 0
                while pending and spent < ns_budget:
                    cost, fn = pending.pop(0)
                    fn()
                    spent += cost

            def drain_all():
                drain(1 << 60)

            def push_qk_group(w, bias_t, dst, c):
                # one psq group: 8 accumulating matmuls + DVE evac w/ bias
                state = {}

                def mk_mm(k):
                    def f():
                        if "ps" not in state:
                            state["ps"] = ps_m.tile([128, 512], F32,
                                                    tag="m512", name="psq")
                        nc.tensor.matmul(
                            state["ps"], w[k],
                            xtr[k][:, c * 512:(c + 1) * 512],
                            start=(k == 0), stop=(k == KT - 1))
                    return f

                for k in range(KT):
                    pending.append((213, mk_mm(k)))

                def evac():
                    nc.vector.tensor_scalar_add(
                        dst[:, c * 512:(c + 1) * 512], state["ps"], bias_t)
                pending.append((0, evac))

            def push_proj_group(ic):
                # proj for the 4 token tiles of query chunk ic
                for tt4 in range(4):
                    state = {}

                    def mk_mm(k, nch, tt4=tt4, state=state):
                        def f():
                            tg = f"mo{nch}"
                            if tg not in state:
                                state[tg] = ps_m.tile([128, 512], F32,
                                                      tag="m512", name="pso")
                            nc.tensor.matmul(
                                state[tg],
                                yT[k][ic][:, tt4 * 128:(tt4 + 1) * 128],
                                wp[k][:, nch * 512:(nch + 1) * 512],
                                start=(k == 0), stop=(k == HPAIRS - 1))
                        return f

                    def mk_evac(nch, tt4=tt4, state=state):
                        def f():
                            tt = ic * 4 + tt4
                            ot = hs.tile([128, 512], F32, tag="ot", bufs=3,
                                         name="ot")
                            nc.vector.tensor_copy(ot, state[f"mo{nch}"])
                            nc.sync.dma_start(
                                out[tt * 128:(tt + 1) * 128,
                                    nch * 512:(nch + 1) * 512], ot)
                        return f

                    for k in range(HPAIRS):
                        for nch in range(2):
                            pending.append((213, mk_mm(k, nch)))
                    for nch in range(2):
                        pending.append((0, mk_evac(nch)))

            # ---- phase V + qk(0): V matmuls interleaved with hp0 q/k ----
            qts = {}
            with tc.tile_pool(name="wvp", bufs=1) as wvp:
                bvr = wvp.tile([1, 512], F32, name="bvr")
                nc.scalar.dma_start(bvr, bv)
                nc.gpsimd.partition_broadcast(bvb, bvr)
                wv = [wvp.tile([128, 512], BF16, name=f"wv{k}")
                      for k in range(KT)]
                for k in range(KT):
                    nc.scalar.dma_start(wv[k], Wv[k * 128:(k + 1) * 128, :])
                w0 = fetch_w(0)
                qt0 = hs.tile([128, S_tok], BF16, tag="qt", bufs=2,
                              name="qt")
                kt0 = hs.tile([128, S_tok], BF16, tag="kt", bufs=2,
                              name="kt")
                qts[0] = (qt0, kt0)
                for c in range(IC):
                    cs = slice(c * 512, (c + 1) * 512)
                    for k in range(KT):
                        nc.sync.dma_start(xtr[k][:, cs],
                                          xT[k * 128:(k + 1) * 128, cs])
                for c in range(IC):
                    for j in range(4):
                        t = c * 4 + j
                        psv = ps_m.tile([128, 512], F32, tag="m512",
                                        name="psv")
                        for k in range(KT):
                            nc.tensor.matmul(
                                psv, xtr[k][:, t * 128:(t + 1) * 128],
                                wv[k], start=(k == 0), stop=(k == KT - 1))
                        psv_r = psv.rearrange("p (hh two c) -> p hh two c",
                                              two=2, c=64)
                        bvb_r = bvb.rearrange("p (hh two c) -> p hh two c",
                                              two=2, c=64)
                        s = t % 2
                        nc.vector.tensor_add(
                            v8[t // 2][:, s:s + 1, 0:1, :, 0:64],
                            psv_r[:, :, 0:1, :], bvb_r[:, :, 0:1, :])
                        nc.vector.tensor_add(
                            v8[t // 2][:, s:s + 1, 1:2, :, 64:128],
                            psv_r[:, :, 1:2, :], bvb_r[:, :, 1:2, :])
                    push_qk_group(w0[0], w0[2], qt0, c)
                    push_qk_group(w0[1], w0[3], kt0, c)
                    drain_all()

            # ---- attention per hp, with spliced qk(hp+1)/proj ----
            for hp in range(HPAIRS):
                qt, kt_ = qts[hp]
                if hp + 1 < HPAIRS:
                    wn = fetch_w(hp + 1)
                    qtn = hs.tile([128, S_tok], BF16, tag="qt", bufs=2,
                                  name="qt")
                    ktn = hs.tile([128, S_tok], BF16, tag="kt", bufs=2,
                                  name="kt")
                    qts[hp + 1] = (qtn, ktn)
                if hp == HPAIRS - 1:
                    for k in range(HPAIRS):
                        nc.scalar.dma_start(wp[k],
                                            Wp[k * 128:(k + 1) * 128, :])
                for ic in range(IC):
                    if hp + 1 < HPAIRS:
                        push_qk_group(wn[0], wn[2], qts[hp + 1][0], ic)
                        push_qk_group(wn[1], wn[3], qts[hp + 1][1], ic)
                    psy = [ps_y.tile([128, 512], F32, tag=f"psy{h2}",
                                     name=f"psy{h2}") for h2 in range(2)]
                    # key-tile order: off-diag ascending, then diag
                    # [tdx1, tdx2, tdx3, tdx0] so the last PV is
                    # full-width (clean stop for the whole psy bank).
                    jts = list(range(4 * ic)) + \
                        [4 * ic + 1, 4 * ic + 2, 4 * ic + 3, 4 * ic]
                    n_jt = len(jts)
                    for sidx, jt in enumerate(jts):
                        g, s2 = jt // 2, jt % 2
                        tdx = jt - 4 * ic
                        off = 128 * tdx if tdx >= 0 else 0
                        qs0 = ic * 512 + off
                        pss = ps_s.tile([128, 1024], F32, tag="pss",
                                        name="pss")
                        nc.tensor.matmul(
                            pss[:, off:512],
                            kt_[0:64, jt * 128:(jt + 1) * 128],
                            qt[0:64, qs0:(ic + 1) * 512],
                            start=True, stop=True,
                            tile_position=(0, 0))
                        nc.tensor.matmul(
                            pss[:, 512 + off:1024],
                            kt_[64:128, jt * 128:(jt + 1) * 128],
                            qt[64:128, qs0:(ic + 1) * 512],
                            start=True, stop=True,
                            tile_position=(64, 0))
                        et = hs.tile([128, 2, 512], BF16, tag="et",
                                     bufs=6, name="et")
                        pss_r = pss.rearrange("p (h q) -> p h q", h=2)
                        nc.scalar.activation(
                            et[:, :, off:512], pss_r[:, :, off:512],
                            ActF.Exp, bias=expb, scale=0.125)
                        if tdx >= 0:
                            ev = et[:, :, off:off + 128]
                            nc.vector.tensor_mul(ev, ev, mask2)
                        for h2 in range(2):
                            nc.tensor.matmul(
                                psy[h2][:, off:512],
                                v8[g][:, s2:s2 + 1, h2:h2 + 1,
                                      hp:hp + 1, :],
                                et[:, h2:h2 + 1, off:512],
                                start=(sidx == 0),
                                stop=(sidx == n_jt - 1))
                        drain(500)
                    # ---- evacuate + normalize this ic ----
                    zc = hs.tile([1, 1024], F32, tag="zc", bufs=2,
                                 name="zc")
                    nc.vector.tensor_copy(yT[hp][ic][0:64, :],
                                          psy[0][0:64, :])
                    nc.vector.tensor_copy(yT[hp][ic][64:128, :],
                                          psy[1][64:128, :])
                    nc.scalar.activation(
                        zc[0:1, 0:512], psy[0][64:65, :], ActF.Copy)
                    nc.scalar.activation(
                        zc[0:1, 512:1024], psy[1][0:1, :], ActF.Copy)
                    zs = hs.tile([128, 8], F32, tag="zs", bufs=2,
                                 name="zs")
                    nc.sync.dma_start(zs, zc)
                    nc.vector.reciprocal(zs, zs)
                    nc.sync.dma_start(zc, zs)
                    bcf = hs.tile([128, 512], F32, tag="bcf", bufs=2,
                                  name="bcf")
                    nc.gpsimd.partition_broadcast(bcf, zc[0:1, 512:1024])
                    nc.gpsimd.partition_broadcast(bcf[0:64, :],
                                                  zc[0:1, 0:512])
                    nc.vector.tensor_mul(yT[hp][ic], yT[hp][ic], bcf)
                    if hp == HPAIRS - 1 and ic >= 1:
                        push_proj_group(ic - 1)
                drain_all()
            push_proj_group(IC - 1)
            drain_all()
    nc.finalize()
    _nc_cache[key] = nc
    return nc


def make_in_maps(x, W_attn, b_attn, W_proj):
    Bx, Sx, Dx = x.shape
    bf = ml_dtypes.bfloat16
    in_maps = []
    xt_b = [np.ascontiguousarray(x[b].T).astype(bf) for b in range(Bx)]
    for c in range(N_CORES):
        b = c // 2
        g = c % 2
        cs = slice(g * 512, (g + 1) * 512)
        in_maps.append({
            "xT": xt_b[b],
            "Wq": np.ascontiguousarray(W_attn[:, 0:D][:, cs]).astype(bf),
            "Wk": np.ascontiguousarray(W_attn[:, D:2 * D][:, cs]).astype(bf),
            "Wv": np.ascontiguousarray(W_attn[:, 2 * D:3 * D][:, cs]).astype(bf),
            "bq": np.ascontiguousarray(
                b_attn[0:D][cs][:, None]).astype(np.float32),
            "bk": np.ascontiguousarray(
                b_attn[D:2 * D][cs][:, None]).astype(np.float32),
            "bv": np.ascontiguousarray(
                b_attn[2 * D:3 * D][cs][None, :]).astype(np.float32),
            "Wp": np.ascontiguousarray(W_proj[cs, :]).astype(bf),
        })
    return in_maps


def kernel(x, W_attn, b_attn, W_proj, b_proj, trace=False):
    x = np.asarray(x, dtype=np.float32)
    W_attn = np.asarray(W_attn, dtype=np.float32)
    b_attn = np.asarray(b_attn, dtype=np.float32)
    W_proj = np.asarray(W_proj, dtype=np.float32)
    b_proj = np.asarray(b_proj, dtype=np.float32)
    nc = build_nc(x.shape[1], N_CORES)
    in_maps = make_in_maps(x, W_attn, b_attn, W_proj)
    res = bass_utils.run_bass_kernel_spmd(
        nc, in_maps, core_ids=list(range(N_CORES)), trace=trace)
    Bx, Sx, Dx = x.shape
    outp = np.empty((Bx, Sx, Dx), dtype=np.float32)
    for b in range(Bx):
        outp[b] = (res.results[2 * b]["out"] + res.results[2 * b + 1]["out"]
                   + b_proj[None, :])
    if trace:
        return outp, res
    return outp
